# revision 1
# baseline (speedup 1.0000x reference)
"""ConditionalFilterLayer Bass/Tile kernel for 8 Trainium2 NeuronCores.

Strategy: pure data parallel over batch (1 sample per core).
Per core, with X = x[i] viewed as [C=512, S=16384] (c-major):
  1. pre_mask[k,s] = Wm @ X; mask = sigmoid(pre_mask + bm)
     (PE with WmT stationary / X moving; bias+sigmoid fused on ACT;
      k padded to 32 so the mask can be xbar-transposed)
  2. class_feat[k,c] = maskT^T @ X^T / S
     (PE; xbar-transposed mask stationary, host-side-transposed xT tiles
      moving; the 1/S is folded into Wf on the host)
  3. filters[k,o] = Wf[k] @ cf[k] + bf  (PE per-class matvec, WfT stationary)
  4. pred[k,s] = filters @ X            (PE; filtersT stationary, X moving)

All matmul inputs bf16 (fp32 PSUM accumulation); measured end-to-end
scale-relative error vs the fp32 reference ~2.7e-3.
"""

import contextlib

import numpy as np
import ml_dtypes

import concourse.bass as bass
import concourse.tile as tile
from concourse import mybir
from concourse.bass_utils import run_bass_kernel_spmd
from concourse.vector_clock import ScopedClock

B, C, K, H, W = 8, 512, 19, 128, 128
KP = 32                      # k padded for xbar transpose
S = H * W                    # 16384
NCT = C // 128               # 4 c-chunks
NXB = 16                     # xc DMA blocks (4096 cols each)
XBW = S // NXB
NTB = 32                     # xT stream DMAs, 4 s-chunks (0.5MB) each
NST = S // 512               # 32 s-tiles for step 1 / step 4
NJ = S // 128                # 128 s-chunks for step 2
MTW = 2048                   # mask xbar transpose width (16 s-chunks)
NMT = S // MTW               # 8 mask transposes
N_CORES = 8

F32 = mybir.dt.float32
BF16 = mybir.dt.bfloat16
npbf16 = ml_dtypes.bfloat16


class TC(tile.TileContext):
    """TileContext whose exit drain carries at most one sync wait per
    instruction — this walrus build rejects multi-wait CTRL ops."""

    def _drain_and_barrier(self, tick_clock, wait_clock):
        nc = self.nc
        drain_inst = nc.sync.drain()
        wait_clock.add_sem_waits(
            drain_inst.ins, ScopedClock({None: tick_clock.global_clock})
        )
        si = drain_inst.ins.sync_info
        waits = list(si.on_wait) if si else []
        if len(waits) > 1:
            SyncInfo = type(si)
            drain_inst.ins.sync_info = SyncInfo(on_wait=[waits[0]], on_update=[])
            for w in waits[1:]:
                n = nc.sync.nop(nofuse=True, hint="split_drain_wait")
                n.ins.sync_info = SyncInfo(on_wait=[w], on_update=[])
        nc.all_engine_barrier()
        assert self.sems is not None
        popped = nc._tile_sem_poison_stack.pop()
        assert popped is self._sem_poison
        nc.clear_and_free_semaphores(list(self.sems.allocated().values()))
        nc.all_engine_barrier()


def _split_multiwaits(nc, max_waits=1):
    """This walrus build rejects instructions with more than one sync wait:
    peel extra waits onto same-engine no-ops inserted just before."""
    import bass_rust
    for f in nc.m.functions:
        for bb in f.blocks:
            insts = list(bb.instructions)
            out, changed = [], False
            for inst in insts:
                si = inst.sync_info
                waits = list(si.on_wait) if si else []
                if len(waits) > max_waits:
                    for w in waits[:-max_waits]:
                        n = mybir.InstNoOp(
                            name=f"I-wsplit-{nc.next_id()}", ins=[], outs=[]
                        )
                        n.engine = inst.engine
                        n.sync_info = bass_rust.SyncInfo(on_wait=[w], on_update=[])
                        out.append(n)
                    inst.sync_info = bass_rust.SyncInfo(
                        on_wait=waits[-max_waits:], on_update=list(si.on_update)
                    )
                    changed = True
                out.append(inst)
            if changed:
                bb.instructions = out


PHASE_MARKS = {}


def _build_kernel():
    nc = bass.Bass("TRN2", target_bir_lowering=False, debug=False)
    PHASE_MARKS.clear()

    def mark(name):
        PHASE_MARKS[name] = nc.next_id()

    xc_d = nc.dram_tensor("xc", [C, S], BF16, kind="ExternalInput").ap()
    xT_d = nc.dram_tensor("xT", [S, C], BF16, kind="ExternalInput").ap()
    wmT_d = nc.dram_tensor("wmT", [NCT, 128, KP], BF16, kind="ExternalInput").ap()
    bm_d = nc.dram_tensor("bm_pad", [128, 1], F32, kind="ExternalInput").ap()
    wfT_d = nc.dram_tensor("wfT", [K, NCT, 128, C], BF16, kind="ExternalInput").ap()
    bfT_d = nc.dram_tensor("bfT", [128, NCT * K], F32, kind="ExternalInput").ap()
    pred_d = nc.dram_tensor("pred", [K, S], F32, kind="ExternalOutput").ap()

    with TC(nc) as tc, contextlib.ExitStack() as ctx:
        const_p = ctx.enter_context(tc.tile_pool(name="const", bufs=1))
        xc_p = ctx.enter_context(tc.tile_pool(name="xc", bufs=1))
        xT_p = ctx.enter_context(tc.tile_pool(name="xT", bufs=5))
        mask_p = ctx.enter_context(tc.tile_pool(name="mask", bufs=1))
        maskT_p = ctx.enter_context(tc.tile_pool(name="maskT", bufs=1))
        cf_p = ctx.enter_context(tc.tile_pool(name="cf", bufs=1))
        cfT_p = ctx.enter_context(tc.tile_pool(name="cfT", bufs=1))
        wf_p = ctx.enter_context(tc.tile_pool(name="wf", bufs=8))
        filt_p = ctx.enter_context(tc.tile_pool(name="filt", bufs=1))
        out_p = ctx.enter_context(tc.tile_pool(name="outp", bufs=3))

        # --- resident x pool + first blocks before consts (head latency) ---
        xc = [[None] * NXB for _ in range(NCT)]

        def emit_xc(blk):
            for ct in range(NCT):
                t = xc_p.tile([128, XBW], BF16, tag=f"xc{ct}_{blk}",
                              name=f"xc{ct}_{blk}")
                nc.sync.dma_start(
                    t[:],
                    xc_d[ct * 128:(ct + 1) * 128, blk * XBW:(blk + 1) * XBW],
                )
                xc[ct][blk] = t

        emit_xc(0)

        # --- constants ---
        wm_sb = const_p.tile([128, NCT * KP], BF16, tag="wm")
        nc.sync.dma_start(wm_sb[:], wmT_d.rearrange("t p k -> p t k"))
        bm_sb = const_p.tile([128, 1], F32, tag="bm")
        nc.sync.dma_start(bm_sb[:], bm_d[:])
        bf_sb = const_p.tile([128, NCT * K], F32, tag="bf")

        def xslice(ct, col0, width):
            blk = col0 // XBW
            off = col0 % XBW
            assert off + width <= XBW
            return xc[ct][blk][:, off:off + width]

        mark("A_start")
        # --- phase A: mask + transpose + class_feat, pipelined per group ---
        # mask_pack[32r+kk, g*512+col] = mask[kk, s], s = (g*4+r)*512 + col
        mask_pack = mask_p.tile([128, NMT * 512], BF16, tag="maskbig")
        NJT = NJ // NTB
        maskT = []
        with tc.tile_pool(name="ps_pm", bufs=4, space="PSUM") as pm_p, \
             tc.tile_pool(name="ps_cf", bufs=1, space="PSUM") as cfps_p:
            cf_ps = cfps_p.tile([KP, C], F32)
            xT_tiles = {}

            def emit_xT(g):
                for tb in range(g * (NTB // NMT), (g + 1) * (NTB // NMT)):
                    xT = xT_p.tile([128, NJT, C], BF16)
                    nc.sync.dma_start(
                        xT[:],
                        xT_d.rearrange("(t p) c -> p t c", p=128)[
                            :, tb * NJT:(tb + 1) * NJT, :
                        ],
                    )
                    xT_tiles[tb] = xT

            def cf_mm(j):
                g, r, n = j // 16, (j % 16) // 4, j % 4
                nc.tensor.matmul(
                    cf_ps[:],
                    lhsT=maskT[g][:, n, 32 * r:32 * r + KP],
                    rhs=xT_tiles[j // NJT][:, j % NJT, :],
                    start=(j == 0),
                    stop=(j == NJ - 1),
                )

            # software pipeline: cf matmuls lag one group behind step 1,
            # interleaved 1:1 so the PE's static order never stalls on the
            # xT stream.
            LAG = 2
            for blk in range(1, NXB):
                emit_xc(blk)
            for gg in range(LAG):
                emit_xT(gg)
            for g in range(NMT):
                if g + LAG < NMT:
                    emit_xT(g + LAG)
                pm = pm_p.tile([128, 512], F32)
                for r in range(4):
                    st = g * 4 + r
                    for ct in range(NCT):
                        nc.tensor.matmul(
                            pm[32 * r:32 * (r + 1), :],
                            lhsT=wm_sb[:, ct * KP:(ct + 1) * KP],
                            rhs=xslice(ct, st * 512, 512),
                            start=(ct == 0),
                            stop=(ct == NCT - 1),
                            tile_position=(0, 32 * r),
                        )
                        if g >= LAG:
                            cf_mm((g - LAG) * 16 + r * 4 + ct)
                nc.scalar.activation(
                    mask_pack[:, g * 512:(g + 1) * 512], pm[:],
                    mybir.ActivationFunctionType.Sigmoid,
                    bias=bm_sb[:],
                )
                mt = maskT_p.tile([128, 4, 128], BF16, tag=f"mT{g}",
                                  name=f"mT{g}")
                nc.sync.dma_start(
                    mt[:], mask_pack[:, g * 512:(g + 1) * 512], transpose=True
                )
                maskT.append(mt)
            for j in range((NMT - LAG) * 16, NJ):
                cf_mm(j)
            cf_sb = cf_p.tile([KP, C], BF16)
            nc.vector.tensor_copy(cf_sb[:], cf_ps[:])

        mark("B_start")
        # --- phase B: cfT via one xbar transpose + per-class filter GEMM ---
        with tc.tile_pool(name="ps_f", bufs=1, space="PSUM") as f_p:
            nc.sync.dma_start(bf_sb[:], bfT_d[:])
            # cfT_all[c_local, ct, kk] = cf[kk, ct*128 + c_local]
            cfT_all = cfT_p.tile([128, NCT, KP], BF16, tag="cfTall")
            nc.scalar.dma_start(cfT_all[:], cf_sb[:], transpose=True)
            cfT = [cfT_all[:, ct, :] for ct in range(NCT)]

            psf = [f_p.tile([128, K], F32, tag=f"psf{oc}", name=f"psf{oc}")
                   for oc in range(NCT)]
            for k in range(K):
                wf_sb = wf_p.tile([128, NCT * C], BF16)
                nc.sync.dma_start(
                    wf_sb[:], wfT_d[k].rearrange("t p c -> p t c")
                )
                for ct in range(NCT):
                    for oc in range(NCT):
                        nc.tensor.matmul(
                            psf[oc][:, k:k + 1],
                            lhsT=wf_sb[:, ct * C + oc * 128:
                                       ct * C + (oc + 1) * 128],
                            rhs=cfT[ct][:, k:k + 1],
                            start=(ct == 0),
                            stop=(ct == NCT - 1),
                        )
            filtT = []
            for oc in range(NCT):
                ft = filt_p.tile([128, K], BF16, tag=f"f{oc}", name=f"f{oc}")
                nc.vector.tensor_add(
                    ft[:], psf[oc][:], bf_sb[:, oc * K:(oc + 1) * K]
                )
                filtT.append(ft)

        mark("D_start")
        # --- phase D: pred = filters @ X ---
        with tc.tile_pool(name="ps_p", bufs=6, space="PSUM") as p_p:
            for blk in range(NST):
                pp = p_p.tile([K, 512], F32)
                for oc in range(NCT):
                    nc.tensor.matmul(
                        pp[:],
                        lhsT=filtT[oc][:],
                        rhs=xslice(oc, blk * 512, 512),
                        start=(oc == 0),
                        stop=(oc == NCT - 1),
                    )
                osb = out_p.tile([K, 512], F32)
                nc.vector.tensor_copy(osb[:], pp[:])
                nc.sync.dma_start(pred_d[:, blk * 512:(blk + 1) * 512], osb[:])

    mark("end")
    _split_multiwaits(nc)
    return nc


_NC_CACHE = None


def _prep_in_maps(x, Wm, bm, Wf, bf):
    wm_pad = np.zeros((C, KP), np.float32)
    wm_pad[:, :K] = Wm.T
    wmT = np.ascontiguousarray(wm_pad.reshape(NCT, 128, KP).astype(npbf16))
    bm_pad = np.zeros((128, 1), np.float32)
    for r in range(4):
        bm_pad[32 * r:32 * r + K, 0] = bm
    # wfT[k, ct, c_local, o] = Wf[k, o, ct*128+c_local] / S  (pool scale folded)
    wfT = np.ascontiguousarray(
        (Wf.transpose(0, 2, 1) / S).reshape(K, NCT, 128, C).astype(npbf16)
    )
    bfT = np.ascontiguousarray(
        bf.T.reshape(NCT, 128, K).transpose(1, 0, 2).reshape(128, NCT * K)
    ).astype(np.float32)
    maps = []
    for i in range(N_CORES):
        xi = np.ascontiguousarray(x[i].reshape(C, S).astype(npbf16))
        xiT = np.ascontiguousarray(xi.T)
        maps.append({
            "xc": xi,
            "xT": xiT,
            "wmT": wmT,
            "bm_pad": bm_pad,
            "wfT": wfT,
            "bfT": bfT,
        })
    return maps


def kernel(x, Wm, bm, Wf, bf):
    global _NC_CACHE
    if _NC_CACHE is None:
        _NC_CACHE = _build_kernel()
    nc = _NC_CACHE

    x = np.asarray(x, dtype=np.float32)
    in_maps = _prep_in_maps(
        x, np.asarray(Wm, np.float32), np.asarray(bm, np.float32),
        np.asarray(Wf, np.float32), np.asarray(bf, np.float32))

    res = run_bass_kernel_spmd(nc, in_maps, list(range(N_CORES)))
    out = np.stack([res.results[i]["pred"].reshape(K, H, W) for i in range(N_CORES)])
    return out.astype(np.float32)


def time_kernel(inputs, iters=20):
    """Steady-state per-launch wall time (ns) with device-resident inputs."""
    import jax
    from jax.sharding import Mesh, PartitionSpec, NamedSharding
    from jax.experimental.shard_map import shard_map
    from concourse import mybir as _mybir
    from concourse.bass2jax import (
        _bass_exec_p, install_neuronx_cc_hook, partition_id_tensor,
    )
    import time as _time

    global _NC_CACHE
    if _NC_CACHE is None:
        _NC_CACHE = _build_kernel()
    nc = _NC_CACHE
    install_neuronx_cc_hook()

    in_maps = _prep_in_maps(
        np.asarray(inputs["x"], np.float32), np.asarray(inputs["Wm"], np.float32),
        np.asarray(inputs["bm"], np.float32), np.asarray(inputs["Wf"], np.float32),
        np.asarray(inputs["bf"], np.float32))

    in_names, out_names, out_avals, zero_outs = [], [], [], []
    pid_name = nc.partition_id_tensor.name if nc.partition_id_tensor else None
    for alloc in nc.m.functions[0].allocations:
        if not isinstance(alloc, _mybir.MemoryLocationSet):
            continue
        name = alloc.memorylocations[0].name
        if alloc.kind == "ExternalInput":
            if name != pid_name:
                in_names.append(name)
        elif alloc.kind == "ExternalOutput":
            shape = tuple(alloc.tensor_shape)
            dt = _mybir.dt.np(alloc.dtype)
            out_names.append(name)
            out_avals.append(jax.core.ShapedArray(shape, dt))
            zero_outs.append(np.zeros(shape, dt))
    n_params = len(in_names)
    all_in_names = in_names + out_names
    if nc.partition_id_tensor is not None:
        all_in_names = all_in_names + [nc.partition_id_tensor.name]

    def _body(*args):
        operands = list(args)
        if nc.partition_id_tensor is not None:
            operands.append(partition_id_tensor())
        outs = _bass_exec_p.bind(
            *operands,
            out_avals=tuple(out_avals),
            in_names=tuple(all_in_names),
            out_names=tuple(out_names),
            lowering_input_output_aliases=(),
            sim_require_finite=True,
            sim_require_nnan=True,
            nc=nc,
        )
        return tuple(outs)

    devices = jax.devices()[:N_CORES]
    mesh = Mesh(np.asarray(devices), ("core",))
    spec = PartitionSpec("core")
    n_outs = len(out_names)
    sharded = jax.jit(
        shard_map(
            _body, mesh=mesh, in_specs=(spec,) * (n_params + n_outs),
            out_specs=(spec,) * n_outs, check_rep=False,
        ),
        keep_unused=True,
    )
    concat_in = [
        np.concatenate([np.asarray(in_maps[c][nm]) for c in range(N_CORES)], axis=0)
        for nm in in_names
    ]
    concat_zeros = [
        np.zeros((N_CORES * z.shape[0], *z.shape[1:]), z.dtype) for z in zero_outs
    ]
    sh = NamedSharding(mesh, spec)
    dev_in = [jax.device_put(a, sh) for a in concat_in + concat_zeros]
    out = sharded(*dev_in)
    jax.block_until_ready(out)
    t0 = _time.perf_counter()
    for _ in range(iters):
        out = sharded(*dev_in)
    jax.block_until_ready(out)
    dt = (_time.perf_counter() - t0) / iters
    return dt * 1e9



# revision 6
# speedup vs baseline: 1.8147x; 1.8147x over previous
"""ConditionalFilterLayer Bass/Tile kernel for 8 Trainium2 NeuronCores.

Strategy: pure data parallel over batch (1 sample per core), all bf16.
Per core, with X = x[i] viewed as [C=512, S=16384] (c-major), X is loaded
from HBM exactly once (16MB bf16) and stays resident in SBUF. Every GEMM
is phrased with an X-chunk as the 128x128 *stationary* operand and a
19-column matrix as the moving operand, so the PE streams only ~19 cycles
per 128x128 contraction tile:

  A1. preT[s,k]  = X-chunk^T @ WmT-chunk (+ ones^T @ bm), sigmoid on ACT
      -> maskT produced directly in [s, k] layout (no mask transpose).
  A2. xT blocks via PE transpose (identity moving), then
      cfT[c,k] += xT-block^T @ maskT-block  (PSUM accumulated over all s)
  B.  psf[o,k]  += Wf-chunk^T @ cfT-column  (per-class filter GEMM),
      + bf -> fT (bf16)
  C.  predT[s,k] = X-chunk^T @ fT-chunk     -> HBM as [S, 19] f32

DMA traffic per core: 16MB x + 10MB Wf + 1.25MB out (vs 43MB baseline).
PSUM outputs are bank-packed (8 s-blocks of 19 cols per 2KB bank) so the
PSUM->SBUF copies are batched; copies rotate across DVE/ACT/GpSimd.
"""

import contextlib

import numpy as np
import ml_dtypes

import concourse.bass as bass
import concourse.tile as tile
from concourse import mybir
from concourse.bass_utils import run_bass_kernel_spmd
from concourse.vector_clock import ScopedClock

B, C, K, H, W = 8, 512, 19, 128, 128
S = H * W                    # 16384
NCT = C // 128               # 4 c-chunks
NSLAB = 4                    # x DMA slabs per c-chunk
SLABW = S // NSLAB           # 4096 s-cols per slab
NBLK = S // 128              # 128 s-blocks / s-chunks
NPAIR = NBLK // 2            # 64 transpose pairs
NGRP = NBLK // 8             # 16 groups of 8 s-blocks (one PSUM bank each)
LAG = 2                      # A2 software pipeline depth (pairs)
N_CORES = 8

F32 = mybir.dt.float32
BF16 = mybir.dt.bfloat16
npbf16 = ml_dtypes.bfloat16


class TC(tile.TileContext):
    """TileContext whose exit drain carries at most one sync wait per
    instruction — this walrus build rejects multi-wait CTRL ops."""

    def _drain_and_barrier(self, tick_clock, wait_clock):
        nc = self.nc
        drain_inst = nc.sync.drain()
        wait_clock.add_sem_waits(
            drain_inst.ins, ScopedClock({None: tick_clock.global_clock})
        )
        si = drain_inst.ins.sync_info
        waits = list(si.on_wait) if si else []
        if len(waits) > 1:
            SyncInfo = type(si)
            drain_inst.ins.sync_info = SyncInfo(on_wait=[waits[0]], on_update=[])
            for w in waits[1:]:
                n = nc.sync.nop(nofuse=True, hint="split_drain_wait")
                n.ins.sync_info = SyncInfo(on_wait=[w], on_update=[])
        nc.all_engine_barrier()
        assert self.sems is not None
        popped = nc._tile_sem_poison_stack.pop()
        assert popped is self._sem_poison
        nc.clear_and_free_semaphores(list(self.sems.allocated().values()))
        nc.all_engine_barrier()


def _split_multiwaits(nc, max_waits=1):
    """This walrus build rejects instructions with more than one sync wait:
    peel extra waits onto same-engine no-ops inserted just before."""
    import bass_rust
    for f in nc.m.functions:
        for bb in f.blocks:
            insts = list(bb.instructions)
            out, changed = [], False
            for inst in insts:
                si = inst.sync_info
                waits = list(si.on_wait) if si else []
                if len(waits) > max_waits:
                    for w in waits[:-max_waits]:
                        n = mybir.InstNoOp(
                            name=f"I-wsplit-{nc.next_id()}", ins=[], outs=[]
                        )
                        n.engine = inst.engine
                        n.sync_info = bass_rust.SyncInfo(on_wait=[w], on_update=[])
                        out.append(n)
                    inst.sync_info = bass_rust.SyncInfo(
                        on_wait=waits[-max_waits:], on_update=list(si.on_update)
                    )
                    changed = True
                out.append(inst)
            if changed:
                bb.instructions = out


PHASE_MARKS = {}


def _build_kernel():
    nc = bass.Bass("TRN2", target_bir_lowering=False, debug=False)
    PHASE_MARKS.clear()

    def mark(name):
        PHASE_MARKS[name] = nc.next_id()

    x_d = nc.dram_tensor("x", [NCT, 128, S], BF16, kind="ExternalInput").ap()
    wmT_d = nc.dram_tensor("wmT", [128, NCT, K], BF16, kind="ExternalInput").ap()
    bm_d = nc.dram_tensor("bm_row", [1, K], BF16, kind="ExternalInput").ap()
    ones_d = nc.dram_tensor("ones_row", [1, 128], BF16, kind="ExternalInput").ap()
    ident_d = nc.dram_tensor("ident", [128, 128], BF16, kind="ExternalInput").ap()
    wfT_d = nc.dram_tensor("wfT", [K, 128, NCT, C], BF16, kind="ExternalInput").ap()
    bfT_d = nc.dram_tensor("bfT", [128, NCT * K], F32, kind="ExternalInput").ap()
    predT_d = nc.dram_tensor("predT", [S, K], F32, kind="ExternalOutput").ap()

    with TC(nc) as tc, contextlib.ExitStack() as ctx:
        const_p = ctx.enter_context(tc.tile_pool(name="const", bufs=1))
        x_p = ctx.enter_context(tc.tile_pool(name="x", bufs=1))
        maskT_p = ctx.enter_context(tc.tile_pool(name="maskT", bufs=1))
        xT_p = ctx.enter_context(tc.tile_pool(name="xT", bufs=6))
        wf_p = ctx.enter_context(tc.tile_pool(name="wf", bufs=7))
        cfT_p = ctx.enter_context(tc.tile_pool(name="cfT", bufs=1))
        fT_p = ctx.enter_context(tc.tile_pool(name="fT", bufs=1))
        stage_p = ctx.enter_context(tc.tile_pool(name="stage", bufs=8))

        # --- constants needed by A1 first, then x slab 0, then the rest ---
        wm_sb = const_p.tile([128, NCT * K], BF16, tag="wm")
        nc.sync.dma_start(wm_sb[:], wmT_d.rearrange("p t k -> p (t k)"))
        ones_sb = const_p.tile([1, 128], BF16, tag="ones")
        nc.sync.dma_start(ones_sb[:], ones_d[:])
        bm_sb = const_p.tile([1, K], BF16, tag="bm")
        nc.sync.dma_start(bm_sb[:], bm_d[:])

        # --- resident x: one tile per (c-chunk, slab) for slice-level deps ---
        x_t = [[None] * NSLAB for _ in range(NCT)]

        def emit_x_slab(sl):
            for ct in range(NCT):
                t = x_p.tile([128, SLABW], BF16, tag=f"x{ct}_{sl}")
                nc.sync.dma_start(
                    t[:], x_d[ct, :, sl * SLABW:(sl + 1) * SLABW]
                )
                x_t[ct][sl] = t

        emit_x_slab(0)
        ident_sb = const_p.tile([128, 128], BF16, tag="ident")
        nc.sync.dma_start(ident_sb[:], ident_d[:])
        bf_sb = const_p.tile([128, NCT * K], F32, tag="bf")
        nc.sync.dma_start(bf_sb[:], bfT_d[:])
        for sl in range(1, NSLAB):
            emit_x_slab(sl)

        def xslice(ct, col0, width=128):
            sl, off = divmod(col0, SLABW)
            assert off + width <= SLABW
            return x_t[ct][sl][:, off:off + width]

        # GPSIMD cannot touch PSUM; rotate PSUM->SBUF copies over DVE (2x
        # bf16 mode) and ACT, weighted toward DVE.
        copy_engines = [
            nc.vector.tensor_copy,
            nc.scalar.copy,
            nc.vector.tensor_copy,
        ]

        mark("A_start")
        # --- phase A: mask (A1) + x-transpose + class_feat (A2), per slab ---
        maskT = []   # 16 tiles [128, 8*19] bf16; (p, b*19+k) = mask[k, (8g+b)*128+p]
        with tc.tile_pool(name="ps_a1", bufs=2, space="PSUM") as a1_ps, \
             tc.tile_pool(name="ps_tr", bufs=5, space="PSUM") as tr_ps, \
             tc.tile_pool(name="ps_cf", bufs=1, space="PSUM") as cf_psp:
            # all 4 cfT accumulators share one bank: a single start marks the
            # bank pending-zero; each column group's first write clears its
            # own bytes, later writes accumulate.
            cf_ps = cf_psp.tile([128, 512], F32, tag="cf", name="cf")
            xt_tiles = {}

            def emit_a2_mm(pair):
                xt = xt_tiles.pop(pair)
                for half in range(2):
                    j = 2 * pair + half
                    g, b = divmod(j, 8)
                    for cb in range(NCT):
                        nc.tensor.matmul(
                            cf_ps[:, K * cb:K * (cb + 1)],
                            lhsT=xt[:, (4 * half + cb) * 128:
                                    (4 * half + cb + 1) * 128],
                            rhs=maskT[g][:, K * b:K * (b + 1)],
                            start=(j == 0 and cb == 0),
                            stop=(j == NBLK - 1 and cb == NCT - 1),
                            skip_group_check=True,
                        )

            for sl in range(NSLAB):
                # A1: mask for this slab (4 groups of 8 s-blocks)
                for g in range(4 * sl, 4 * (sl + 1)):
                    ps = a1_ps.tile([128, 512], F32)
                    for b in range(8):
                        blk = 8 * g + b
                        for ct in range(NCT):
                            nc.tensor.matmul(
                                ps[:, K * b:K * (b + 1)],
                                lhsT=xslice(ct, blk * 128),
                                rhs=wm_sb[:, K * ct:K * (ct + 1)],
                                start=(b == 0 and ct == 0),
                                stop=False,
                                skip_group_check=True,
                            )
                        nc.tensor.matmul(
                            ps[:, K * b:K * (b + 1)],
                            lhsT=ones_sb[:],
                            rhs=bm_sb[:],
                            start=False,
                            stop=(b == 7),
                            skip_group_check=True,
                        )
                    mt = maskT_p.tile([128, 8 * K], BF16, tag=f"mT{g}",
                                      name=f"mT{g}")
                    nc.scalar.activation(
                        mt[:], ps[:, :8 * K],
                        mybir.ActivationFunctionType.Sigmoid,
                    )
                    maskT.append(mt)
                # A2: transpose x blocks + accumulate cfT (lag pipeline)
                for pair in range(16 * sl, 16 * (sl + 1)):
                    tp = tr_ps.tile([128, 1024], BF16)
                    for half in range(2):
                        j = 2 * pair + half
                        for cb in range(NCT):
                            nc.tensor.matmul(
                                tp[:, (4 * half + cb) * 128:
                                   (4 * half + cb + 1) * 128],
                                lhsT=xslice(cb, j * 128),
                                rhs=ident_sb[:],
                                is_transpose=True,
                                start=True, stop=True,
                                skip_group_check=True,
                            )
                    xt = xT_p.tile([128, 1024], BF16)
                    copy_engines[pair % 3](xt[:], tp[:])
                    xt_tiles[pair] = xt
                    if pair >= LAG:
                        emit_a2_mm(pair - LAG)
            for pair in range(NPAIR - LAG, NPAIR):
                emit_a2_mm(pair)

            cfT_sb = cfT_p.tile([128, NCT * K], BF16, tag="cfT")
            nc.vector.tensor_copy(cfT_sb[:], cf_ps[:, :NCT * K])

        mark("B_start")
        # --- phase B: per-class filter GEMM; fT[c, k] ready for step C ---
        with tc.tile_pool(name="ps_f", bufs=1, space="PSUM") as f_psp:
            psf = [f_psp.tile([128, 512], F32, tag=f"psf{oc}",
                              name=f"psf{oc}") for oc in range(NCT)]
            for kk in range(K):
                wf = wf_p.tile([128, NCT, C], BF16)
                nc.sync.dma_start(wf[:], wfT_d[kk])
                for ct in range(NCT):
                    for oc in range(NCT):
                        nc.tensor.matmul(
                            psf[oc][:, kk:kk + 1],
                            lhsT=wf[:, ct, oc * 128:(oc + 1) * 128],
                            rhs=cfT_sb[:, K * ct + kk:K * ct + kk + 1],
                            start=(ct == 0),
                            stop=(ct == NCT - 1),
                            skip_group_check=True,
                        )
            fT_sb = fT_p.tile([128, NCT * K], BF16, tag="fT")
            for oc in range(NCT):
                nc.vector.tensor_add(
                    fT_sb[:, K * oc:K * (oc + 1)],
                    psf[oc][:, :K],
                    bf_sb[:, K * oc:K * (oc + 1)],
                )

        mark("C_start")
        # --- phase C: predT[s, k] = X^T @ fT, bank-packed like A1 ---
        pout = predT_d.rearrange("(m p) k -> p m k", p=128)
        with tc.tile_pool(name="ps_c", bufs=4, space="PSUM") as c_ps:
            for g in range(NGRP):
                ps = c_ps.tile([128, 512], F32)
                for b in range(8):
                    blk = 8 * g + b
                    for ct in range(NCT):
                        nc.tensor.matmul(
                            ps[:, K * b:K * (b + 1)],
                            lhsT=xslice(ct, blk * 128),
                            rhs=fT_sb[:, K * ct:K * (ct + 1)],
                            start=(b == 0 and ct == 0),
                            stop=(b == 7 and ct == NCT - 1),
                            skip_group_check=True,
                        )
                st = stage_p.tile([128, 8, K], F32)
                nc.vector.tensor_copy(
                    st[:].rearrange("p a b -> p (a b)"), ps[:, :8 * K]
                )
                nc.sync.dma_start(pout[:, 8 * g:8 * (g + 1), :], st[:])

    mark("end")
    _split_multiwaits(nc)
    return nc


_NC_CACHE = None


def _prep_in_maps(x, Wm, bm, Wf, bf):
    x = np.asarray(x, np.float32)
    wmT = np.ascontiguousarray(
        Wm.T.reshape(NCT, 128, K).transpose(1, 0, 2)
    ).astype(npbf16)
    bm_row = np.ascontiguousarray(bm.reshape(1, K)).astype(npbf16)
    ones_row = np.ones((1, 128), npbf16)
    ident = np.eye(128, dtype=npbf16)
    # wfT[k, c_local, ct, o] = Wf[k, o, 128*ct + c_local] / S  (pool scale folded)
    wfT = np.ascontiguousarray(
        (Wf.transpose(0, 2, 1) / S).reshape(K, NCT, 128, C).transpose(0, 2, 1, 3)
    ).astype(npbf16)
    bfT = np.ascontiguousarray(
        bf.T.reshape(NCT, 128, K).transpose(1, 0, 2).reshape(128, NCT * K)
    ).astype(np.float32)
    maps = []
    for i in range(N_CORES):
        xi = np.ascontiguousarray(
            x[i].reshape(NCT, 128, S).astype(npbf16)
        )
        maps.append({
            "x": xi,
            "wmT": wmT,
            "bm_row": bm_row,
            "ones_row": ones_row,
            "ident": ident,
            "wfT": wfT,
            "bfT": bfT,
        })
    return maps


def kernel(x, Wm, bm, Wf, bf):
    global _NC_CACHE
    if _NC_CACHE is None:
        _NC_CACHE = _build_kernel()
    nc = _NC_CACHE

    in_maps = _prep_in_maps(
        np.asarray(x, np.float32), np.asarray(Wm, np.float32),
        np.asarray(bm, np.float32), np.asarray(Wf, np.float32),
        np.asarray(bf, np.float32))

    res = run_bass_kernel_spmd(nc, in_maps, list(range(N_CORES)))
    out = np.stack([
        res.results[i]["predT"].T.reshape(K, H, W) for i in range(N_CORES)
    ])
    return out.astype(np.float32)


def time_kernel(inputs, iters=20):
    """Steady-state per-launch wall time (ns) with device-resident inputs."""
    import jax
    from jax.sharding import Mesh, PartitionSpec, NamedSharding
    from jax.experimental.shard_map import shard_map
    from concourse import mybir as _mybir
    from concourse.bass2jax import (
        _bass_exec_p, install_neuronx_cc_hook, partition_id_tensor,
    )
    import time as _time

    global _NC_CACHE
    if _NC_CACHE is None:
        _NC_CACHE = _build_kernel()
    nc = _NC_CACHE
    install_neuronx_cc_hook()

    in_maps = _prep_in_maps(
        np.asarray(inputs["x"], np.float32), np.asarray(inputs["Wm"], np.float32),
        np.asarray(inputs["bm"], np.float32), np.asarray(inputs["Wf"], np.float32),
        np.asarray(inputs["bf"], np.float32))

    in_names, out_names, out_avals, zero_outs = [], [], [], []
    pid_name = nc.partition_id_tensor.name if nc.partition_id_tensor else None
    for alloc in nc.m.functions[0].allocations:
        if not isinstance(alloc, _mybir.MemoryLocationSet):
            continue
        name = alloc.memorylocations[0].name
        if alloc.kind == "ExternalInput":
            if name != pid_name:
                in_names.append(name)
        elif alloc.kind == "ExternalOutput":
            shape = tuple(alloc.tensor_shape)
            dt = _mybir.dt.np(alloc.dtype)
            out_names.append(name)
            out_avals.append(jax.core.ShapedArray(shape, dt))
            zero_outs.append(np.zeros(shape, dt))
    n_params = len(in_names)
    all_in_names = in_names + out_names
    if nc.partition_id_tensor is not None:
        all_in_names = all_in_names + [nc.partition_id_tensor.name]

    def _body(*args):
        operands = list(args)
        if nc.partition_id_tensor is not None:
            operands.append(partition_id_tensor())
        outs = _bass_exec_p.bind(
            *operands,
            out_avals=tuple(out_avals),
            in_names=tuple(all_in_names),
            out_names=tuple(out_names),
            lowering_input_output_aliases=(),
            sim_require_finite=True,
            sim_require_nnan=True,
            nc=nc,
        )
        return tuple(outs)

    devices = jax.devices()[:N_CORES]
    mesh = Mesh(np.asarray(devices), ("core",))
    spec = PartitionSpec("core")
    n_outs = len(out_names)
    sharded = jax.jit(
        shard_map(
            _body, mesh=mesh, in_specs=(spec,) * (n_params + n_outs),
            out_specs=(spec,) * n_outs, check_rep=False,
        ),
        keep_unused=True,
    )
    concat_in = [
        np.concatenate([np.asarray(in_maps[c][nm]) for c in range(N_CORES)], axis=0)
        for nm in in_names
    ]
    concat_zeros = [
        np.zeros((N_CORES * z.shape[0], *z.shape[1:]), z.dtype) for z in zero_outs
    ]
    sh = NamedSharding(mesh, spec)
    dev_in = [jax.device_put(a, sh) for a in concat_in + concat_zeros]
    out = sharded(*dev_in)
    jax.block_until_ready(out)
    t0 = _time.perf_counter()
    for _ in range(iters):
        out = sharded(*dev_in)
    jax.block_until_ready(out)
    dt = (_time.perf_counter() - t0) / iters
    return dt * 1e9


# revision 16
# speedup vs baseline: 2.2372x; 1.2328x over previous
"""ConditionalFilterLayer Bass/Tile kernel for 8 Trainium2 NeuronCores.

Strategy: pure data parallel over batch (1 sample per core), all bf16.
Per core, with X = x[i] viewed as [C=512, S=16384] (c-major), X is loaded
from HBM exactly once (16MB bf16) and stays resident in SBUF. Every GEMM
is phrased with an X-chunk as the 128x128 *stationary* operand and a
19-column matrix as the moving operand, so the PE streams only ~19 cycles
per 128x128 contraction tile:

  A1. preT[s,k]  = X-chunk^T @ WmT-chunk (+ ones^T @ bm), sigmoid on ACT
      -> maskT produced directly in [s, k] layout (no mask transpose).
  A2. xT blocks via PE transpose (identity moving), then
      cfT[c,k] += xT-block^T @ maskT-block  (PSUM accumulated over all s)
  B.  psf[o,k]  += Wf-chunk^T @ cfT-column  (per-class filter GEMM),
      + bf -> fT (bf16)
  C.  predT[s,k] = X-chunk^T @ fT-chunk     -> HBM as [S, 19] f32

DMA traffic per core: 16MB x + 10MB Wf + 1.25MB out (vs 43MB baseline).
PSUM outputs are bank-packed (8 s-blocks of 19 cols per 2KB bank) so the
PSUM->SBUF copies are batched; copies rotate across DVE/ACT/GpSimd.
"""

import contextlib

import numpy as np
import ml_dtypes

import concourse.bass as bass
import concourse.tile as tile
from concourse import mybir
from concourse.bass_utils import run_bass_kernel_spmd
from concourse.vector_clock import ScopedClock

B, C, K, H, W = 8, 512, 19, 128, 128
S = H * W                    # 16384
NCT = C // 128               # 4 c-chunks
NSLAB = 8                    # x DMA slabs per c-chunk
SLABW = S // NSLAB           # 2048 s-cols per slab
NBLK = S // 128              # 128 s-blocks / s-chunks
NPAIR = NBLK // 2            # 64 transpose pairs
NGRP = NBLK // 8             # 16 groups of 8 s-blocks (one PSUM bank each)
LAG = 2                      # A2 software pipeline depth (pairs)
N_CORES = 8

F32 = mybir.dt.float32
BF16 = mybir.dt.bfloat16
F8 = mybir.dt.float8e4
npbf16 = ml_dtypes.bfloat16
npf8 = ml_dtypes.float8_e4m3
WF_SCALE = 16.0              # lifts Wf into fp8 e4m3 normal range


class TC(tile.TileContext):
    """TileContext whose exit drain carries at most one sync wait per
    instruction — this walrus build rejects multi-wait CTRL ops."""

    def _drain_and_barrier(self, tick_clock, wait_clock):
        nc = self.nc
        drain_inst = nc.sync.drain()
        wait_clock.add_sem_waits(
            drain_inst.ins, ScopedClock({None: tick_clock.global_clock})
        )
        si = drain_inst.ins.sync_info
        waits = list(si.on_wait) if si else []
        if len(waits) > 1:
            SyncInfo = type(si)
            drain_inst.ins.sync_info = SyncInfo(on_wait=[waits[0]], on_update=[])
            for w in waits[1:]:
                n = nc.sync.nop(nofuse=True, hint="split_drain_wait")
                n.ins.sync_info = SyncInfo(on_wait=[w], on_update=[])
        nc.all_engine_barrier()
        assert self.sems is not None
        popped = nc._tile_sem_poison_stack.pop()
        assert popped is self._sem_poison
        nc.clear_and_free_semaphores(list(self.sems.allocated().values()))
        nc.all_engine_barrier()


def _split_multiwaits(nc, max_waits=1):
    """This walrus build rejects instructions with more than one sync wait:
    peel extra waits onto same-engine no-ops inserted just before."""
    import bass_rust
    for f in nc.m.functions:
        for bb in f.blocks:
            insts = list(bb.instructions)
            out, changed = [], False
            for inst in insts:
                si = inst.sync_info
                waits = list(si.on_wait) if si else []
                if len(waits) > max_waits:
                    for w in waits[:-max_waits]:
                        n = mybir.InstNoOp(
                            name=f"I-wsplit-{nc.next_id()}", ins=[], outs=[]
                        )
                        n.engine = inst.engine
                        n.sync_info = bass_rust.SyncInfo(on_wait=[w], on_update=[])
                        out.append(n)
                    inst.sync_info = bass_rust.SyncInfo(
                        on_wait=waits[-max_waits:], on_update=list(si.on_update)
                    )
                    changed = True
                out.append(inst)
            if changed:
                bb.instructions = out


PHASE_MARKS = {}


def _build_kernel():
    nc = bass.Bass("TRN2", target_bir_lowering=False, debug=False)
    PHASE_MARKS.clear()

    def mark(name):
        PHASE_MARKS[name] = nc.next_id()

    x_d = nc.dram_tensor("x", [NCT, 128, S], BF16, kind="ExternalInput").ap()
    # packed bf16 consts: [:, 0:76] wmT, [:, 76:204] identity,
    # row0 204:223 bm, row0 223:351 ones
    constA_d = nc.dram_tensor("constA", [128, 352], BF16, kind="ExternalInput").ap()
    wfT_d = nc.dram_tensor("wfT", [K, 128, NCT, C], F8, kind="ExternalInput").ap()
    bfT_d = nc.dram_tensor("bfT", [128, NCT * K], F32, kind="ExternalInput").ap()
    predT_d = nc.dram_tensor("predT", [S, K], F32, kind="ExternalOutput").ap()

    with TC(nc) as tc, contextlib.ExitStack() as ctx:
        const_p = ctx.enter_context(tc.tile_pool(name="const", bufs=1))
        x_p = ctx.enter_context(tc.tile_pool(name="x", bufs=1))
        maskT_p = ctx.enter_context(tc.tile_pool(name="maskT", bufs=1))
        xT_p = ctx.enter_context(tc.tile_pool(name="xT", bufs=4))
        wf_p = ctx.enter_context(tc.tile_pool(name="wf", bufs=14))
        cfT_p = ctx.enter_context(tc.tile_pool(name="cfT", bufs=1))
        fT_p = ctx.enter_context(tc.tile_pool(name="fT", bufs=1))
        stage_p = ctx.enter_context(tc.tile_pool(name="stage", bufs=8))

        # --- packed bf16 consts in one DMA, then x slabs ---
        constA = const_p.tile([128, 352], BF16, tag="constA")
        nc.sync.dma_start(constA[:], constA_d[:])
        wm_sb = constA[:, 0:NCT * K]
        ident_sb = constA[:, 76:204]
        bm_sb = constA[0:1, 204:204 + K]
        ones_sb = constA[0:1, 223:351]

        # --- resident x: one tile per (c-chunk, slab) for slice-level deps ---
        x_t = [[None] * NSLAB for _ in range(NCT)]

        def emit_x_slab(sl):
            for ct in range(NCT):
                t = x_p.tile([128, SLABW], BF16, tag=f"x{ct}_{sl}")
                nc.sync.dma_start(
                    t[:], x_d[ct, :, sl * SLABW:(sl + 1) * SLABW]
                )
                x_t[ct][sl] = t

        emit_x_slab(0)
        bf_sb = const_p.tile([128, NCT * K], F32, tag="bf")
        nc.sync.dma_start(bf_sb[:], bfT_d[:])
        for sl in range(1, NSLAB):
            emit_x_slab(sl)

        def xslice(ct, col0, width=128):
            sl, off = divmod(col0, SLABW)
            assert off + width <= SLABW
            return x_t[ct][sl][:, off:off + width]

        # GPSIMD cannot touch PSUM; rotate PSUM->SBUF copies over DVE (2x
        # bf16 mode) and ACT, weighted toward DVE.
        copy_engines = [
            nc.vector.tensor_copy,
            nc.scalar.copy,
            nc.vector.tensor_copy,
        ]

        mark("A_start")
        # --- phase A: mask (A1) + x-transpose + class_feat (A2), per slab ---
        maskT = []   # 16 tiles [128, 8*19] bf16; (p, b*19+k) = mask[k, (8g+b)*128+p]
        with tc.tile_pool(name="ps_a1", bufs=2, space="PSUM") as a1_ps, \
             tc.tile_pool(name="ps_tr", bufs=5, space="PSUM") as tr_ps, \
             tc.tile_pool(name="ps_cf", bufs=1, space="PSUM") as cf_psp:
            # all 4 cfT accumulators share one bank: a single start marks the
            # bank pending-zero; each column group's first write clears its
            # own bytes, later writes accumulate.
            cf_ps = cf_psp.tile([128, 512], F32, tag="cf", name="cf")
            xt_tiles = {}

            def emit_a2_mm(pair):
                xt = xt_tiles.pop(pair)
                for half in range(2):
                    j = 2 * pair + half
                    g, b = divmod(j, 8)
                    for cb in range(NCT):
                        nc.tensor.matmul(
                            cf_ps[:, K * cb:K * (cb + 1)],
                            lhsT=xt[:, (4 * half + cb) * 128:
                                    (4 * half + cb + 1) * 128],
                            rhs=maskT[g][:, K * b:K * (b + 1)],
                            start=(j == 0 and cb == 0),
                            stop=(j == NBLK - 1 and cb == NCT - 1),
                            skip_group_check=True,
                        )

            for sl in range(NSLAB):
                # A1: mask for this slab (4 groups of 8 s-blocks)
                for g in range(4 * sl, 4 * (sl + 1)):
                    ps = a1_ps.tile([128, 512], F32)
                    for b in range(8):
                        blk = 8 * g + b
                        for ct in range(NCT):
                            nc.tensor.matmul(
                                ps[:, K * b:K * (b + 1)],
                                lhsT=xslice(ct, blk * 128),
                                rhs=wm_sb[:, K * ct:K * (ct + 1)],
                                start=(b == 0 and ct == 0),
                                stop=False,
                                skip_group_check=True,
                            )
                        nc.tensor.matmul(
                            ps[:, K * b:K * (b + 1)],
                            lhsT=ones_sb,
                            rhs=bm_sb,
                            start=False,
                            stop=(b == 7),
                            skip_group_check=True,
                        )
                    mt = maskT_p.tile([128, 8 * K], BF16, tag=f"mT{g}",
                                      name=f"mT{g}")
                    nc.scalar.activation(
                        mt[:], ps[:, :8 * K],
                        mybir.ActivationFunctionType.Sigmoid,
                    )
                    maskT.append(mt)
                # A2: transpose x blocks + accumulate cfT (lag pipeline)
                for pair in range(16 * sl, 16 * (sl + 1)):
                    tp = tr_ps.tile([128, 1024], BF16)
                    for half in range(2):
                        j = 2 * pair + half
                        for cb in range(NCT):
                            nc.tensor.matmul(
                                tp[:, (4 * half + cb) * 128:
                                   (4 * half + cb + 1) * 128],
                                lhsT=xslice(cb, j * 128),
                                rhs=ident_sb,
                                is_transpose=True,
                                start=True, stop=True,
                                skip_group_check=True,
                            )
                    xt = xT_p.tile([128, 1024], BF16)
                    copy_engines[pair % 3](xt[:], tp[:])
                    xt_tiles[pair] = xt
                    if pair >= LAG:
                        emit_a2_mm(pair - LAG)
            for pair in range(NPAIR - LAG, NPAIR):
                emit_a2_mm(pair)

            # fold pooling 1/S and the fp8 Wf scale into cfT
            cfT_sb = cfT_p.tile([128, NCT * K], BF16, tag="cfT")
            nc.vector.tensor_scalar_mul(
                cfT_sb[:], cf_ps[:, :NCT * K], 1.0 / (WF_SCALE * S)
            )

        mark("B_start")
        # --- phase B: per-class filter GEMM; fT[c, k] ready for step C ---
        with tc.tile_pool(name="ps_f", bufs=1, space="PSUM") as f_psp:
            psf = [f_psp.tile([128, 512], F32, tag=f"psf{oc}",
                              name=f"psf{oc}") for oc in range(NCT)]
            for kk in range(K):
                wf = wf_p.tile([128, NCT, C], F8)
                nc.sync.dma_start(wf[:], wfT_d[kk])
                for ct in range(NCT):
                    for oc in range(NCT):
                        nc.tensor.matmul(
                            psf[oc][:, kk:kk + 1],
                            lhsT=wf[:, ct, oc * 128:(oc + 1) * 128],
                            rhs=cfT_sb[:, K * ct + kk:K * ct + kk + 1],
                            start=(ct == 0),
                            stop=(ct == NCT - 1),
                            skip_group_check=True,
                        )
            fT_sb = fT_p.tile([128, NCT * K], BF16, tag="fT")
            for oc in range(NCT):
                nc.vector.tensor_add(
                    fT_sb[:, K * oc:K * (oc + 1)],
                    psf[oc][:, :K],
                    bf_sb[:, K * oc:K * (oc + 1)],
                )

        mark("C_start")
        # --- phase C: predT[s, k] = X^T @ fT, bank-packed like A1 ---
        pout = predT_d.rearrange("(m p) k -> p m k", p=128)
        with tc.tile_pool(name="ps_c", bufs=4, space="PSUM") as c_ps:
            for gg in range(NGRP // 2):
                st = stage_p.tile([128, 16, K], F32)
                for h in range(2):
                    g = 2 * gg + h
                    ps = c_ps.tile([128, 512], F32)
                    for b in range(8):
                        blk = 8 * g + b
                        for ct in range(NCT):
                            nc.tensor.matmul(
                                ps[:, K * b:K * (b + 1)],
                                lhsT=xslice(ct, blk * 128),
                                rhs=fT_sb[:, K * ct:K * (ct + 1)],
                                start=(b == 0 and ct == 0),
                                stop=(b == 7 and ct == NCT - 1),
                                skip_group_check=True,
                            )
                    nc.vector.tensor_copy(
                        st[:, 8 * h:8 * (h + 1), :].rearrange("p a b -> p (a b)"),
                        ps[:, :8 * K],
                    )
                nc.sync.dma_start(pout[:, 16 * gg:16 * (gg + 1), :], st[:])

    mark("end")
    _split_multiwaits(nc)
    return nc


_NC_CACHE = None


def _prep_in_maps(x, Wm, bm, Wf, bf):
    x = np.asarray(x, np.float32)
    constA = np.zeros((128, 352), npbf16)
    # wmT[c_local, ct*K + k] = Wm[k, 128*ct + c_local]
    constA[:, 0:NCT * K] = (
        Wm.T.reshape(NCT, 128, K).transpose(1, 0, 2).reshape(128, NCT * K)
    ).astype(npbf16)
    constA[:, 76:204] = np.eye(128, dtype=npbf16)
    constA[0, 204:204 + K] = bm.astype(npbf16)
    constA[0, 223:351] = np.ones(128, npbf16)
    # wfT[k, c_local, ct, o] = Wf[k, o, 128*ct + c_local] * WF_SCALE (fp8;
    # the 1/(WF_SCALE*S) is folded into cfT on-chip)
    wfT = np.ascontiguousarray(
        (Wf.transpose(0, 2, 1) * WF_SCALE)
        .reshape(K, NCT, 128, C).transpose(0, 2, 1, 3)
    ).astype(npf8)
    bfT = np.ascontiguousarray(
        bf.T.reshape(NCT, 128, K).transpose(1, 0, 2).reshape(128, NCT * K)
    ).astype(np.float32)
    maps = []
    for i in range(N_CORES):
        xi = np.ascontiguousarray(
            x[i].reshape(NCT, 128, S).astype(npbf16)
        )
        maps.append({
            "x": xi,
            "constA": constA,
            "wfT": wfT,
            "bfT": bfT,
        })
    return maps


def kernel(x, Wm, bm, Wf, bf):
    global _NC_CACHE
    if _NC_CACHE is None:
        _NC_CACHE = _build_kernel()
    nc = _NC_CACHE

    in_maps = _prep_in_maps(
        np.asarray(x, np.float32), np.asarray(Wm, np.float32),
        np.asarray(bm, np.float32), np.asarray(Wf, np.float32),
        np.asarray(bf, np.float32))

    res = run_bass_kernel_spmd(nc, in_maps, list(range(N_CORES)))
    out = np.stack([
        res.results[i]["predT"].T.reshape(K, H, W) for i in range(N_CORES)
    ])
    return out.astype(np.float32)


def time_kernel(inputs, iters=20):
    """Steady-state per-launch wall time (ns) with device-resident inputs."""
    import jax
    from jax.sharding import Mesh, PartitionSpec, NamedSharding
    from jax.experimental.shard_map import shard_map
    from concourse import mybir as _mybir
    from concourse.bass2jax import (
        _bass_exec_p, install_neuronx_cc_hook, partition_id_tensor,
    )
    import time as _time

    global _NC_CACHE
    if _NC_CACHE is None:
        _NC_CACHE = _build_kernel()
    nc = _NC_CACHE
    install_neuronx_cc_hook()

    in_maps = _prep_in_maps(
        np.asarray(inputs["x"], np.float32), np.asarray(inputs["Wm"], np.float32),
        np.asarray(inputs["bm"], np.float32), np.asarray(inputs["Wf"], np.float32),
        np.asarray(inputs["bf"], np.float32))

    in_names, out_names, out_avals, zero_outs = [], [], [], []
    pid_name = nc.partition_id_tensor.name if nc.partition_id_tensor else None
    for alloc in nc.m.functions[0].allocations:
        if not isinstance(alloc, _mybir.MemoryLocationSet):
            continue
        name = alloc.memorylocations[0].name
        if alloc.kind == "ExternalInput":
            if name != pid_name:
                in_names.append(name)
        elif alloc.kind == "ExternalOutput":
            shape = tuple(alloc.tensor_shape)
            dt = _mybir.dt.np(alloc.dtype)
            out_names.append(name)
            out_avals.append(jax.core.ShapedArray(shape, dt))
            zero_outs.append(np.zeros(shape, dt))
    n_params = len(in_names)
    all_in_names = in_names + out_names
    if nc.partition_id_tensor is not None:
        all_in_names = all_in_names + [nc.partition_id_tensor.name]

    def _body(*args):
        operands = list(args)
        if nc.partition_id_tensor is not None:
            operands.append(partition_id_tensor())
        outs = _bass_exec_p.bind(
            *operands,
            out_avals=tuple(out_avals),
            in_names=tuple(all_in_names),
            out_names=tuple(out_names),
            lowering_input_output_aliases=(),
            sim_require_finite=True,
            sim_require_nnan=True,
            nc=nc,
        )
        return tuple(outs)

    devices = jax.devices()[:N_CORES]
    mesh = Mesh(np.asarray(devices), ("core",))
    spec = PartitionSpec("core")
    n_outs = len(out_names)
    sharded = jax.jit(
        shard_map(
            _body, mesh=mesh, in_specs=(spec,) * (n_params + n_outs),
            out_specs=(spec,) * n_outs, check_rep=False,
        ),
        keep_unused=True,
    )
    concat_in = [
        np.concatenate([np.asarray(in_maps[c][nm]) for c in range(N_CORES)], axis=0)
        for nm in in_names
    ]
    concat_zeros = [
        np.zeros((N_CORES * z.shape[0], *z.shape[1:]), z.dtype) for z in zero_outs
    ]
    sh = NamedSharding(mesh, spec)
    dev_in = [jax.device_put(a, sh) for a in concat_in + concat_zeros]
    out = sharded(*dev_in)
    jax.block_until_ready(out)
    t0 = _time.perf_counter()
    for _ in range(iters):
        out = sharded(*dev_in)
    jax.block_until_ready(out)
    dt = (_time.perf_counter() - t0) / iters
    return dt * 1e9


# revision 26
# speedup vs baseline: 2.3792x; 1.0635x over previous
"""ConditionalFilterLayer Bass/Tile kernel for 8 Trainium2 NeuronCores.

Strategy: pure data parallel over batch (1 sample per core), all bf16.
Per core, with X = x[i] viewed as [C=512, S=16384] (c-major), X is loaded
from HBM exactly once (16MB bf16) and stays resident in SBUF. Every GEMM
is phrased with an X-chunk as the 128x128 *stationary* operand and a
19-column matrix as the moving operand, so the PE streams only ~19 cycles
per 128x128 contraction tile:

  A1. preT[s,k]  = X-chunk^T @ WmT-chunk (+ ones^T @ bm), sigmoid on ACT
      -> maskT produced directly in [s, k] layout (no mask transpose).
  A2. xT blocks via PE transpose (identity moving), then
      cfT[c,k] += xT-block^T @ maskT-block  (PSUM accumulated over all s)
  B.  psf[o,k]  += Wf-chunk^T @ cfT-column  (per-class filter GEMM),
      + bf -> fT (bf16)
  C.  predT[s,k] = X-chunk^T @ fT-chunk     -> HBM as [S, 19] f32

DMA traffic per core: 16MB x + 10MB Wf + 1.25MB out (vs 43MB baseline).
PSUM outputs are bank-packed (8 s-blocks of 19 cols per 2KB bank) so the
PSUM->SBUF copies are batched; copies rotate across DVE/ACT/GpSimd.
"""

import contextlib

import numpy as np
import ml_dtypes

import concourse.bass as bass
import concourse.tile as tile
from concourse import mybir
from concourse.bass_utils import run_bass_kernel_spmd
from concourse.vector_clock import ScopedClock

B, C, K, H, W = 8, 512, 19, 128, 128
S = H * W                    # 16384
NCT = C // 128               # 4 c-chunks
NSLAB = 8                    # x DMA slabs per c-chunk
SLABW = S // NSLAB           # 2048 s-cols per slab
NBLK = S // 128              # 128 s-blocks / s-chunks
NPAIR = NBLK // 2            # 64 transpose pairs
NGRP = NBLK // 8             # 16 groups of 8 s-blocks (one PSUM bank each)
LAG = 3                      # A2 software pipeline depth (pairs)
N_CORES = 8

F32 = mybir.dt.float32
BF16 = mybir.dt.bfloat16
F8 = mybir.dt.float8e4
npbf16 = ml_dtypes.bfloat16
npf8 = ml_dtypes.float8_e4m3
WF_SCALE = 16.0              # lifts Wf into fp8 e4m3 normal range


class TC(tile.TileContext):
    """TileContext whose exit drain carries at most one sync wait per
    instruction — this walrus build rejects multi-wait CTRL ops."""

    def _drain_and_barrier(self, tick_clock, wait_clock):
        nc = self.nc
        drain_inst = nc.sync.drain()
        wait_clock.add_sem_waits(
            drain_inst.ins, ScopedClock({None: tick_clock.global_clock})
        )
        si = drain_inst.ins.sync_info
        waits = list(si.on_wait) if si else []
        if len(waits) > 1:
            SyncInfo = type(si)
            drain_inst.ins.sync_info = SyncInfo(on_wait=[waits[0]], on_update=[])
            for w in waits[1:]:
                n = nc.sync.nop(nofuse=True, hint="split_drain_wait")
                n.ins.sync_info = SyncInfo(on_wait=[w], on_update=[])
        nc.all_engine_barrier()
        assert self.sems is not None
        popped = nc._tile_sem_poison_stack.pop()
        assert popped is self._sem_poison
        nc.clear_and_free_semaphores(list(self.sems.allocated().values()))
        nc.all_engine_barrier()


def _split_multiwaits(nc, max_waits=1):
    """This walrus build rejects instructions with more than one sync wait:
    peel extra waits onto same-engine no-ops inserted just before."""
    import bass_rust
    for f in nc.m.functions:
        for bb in f.blocks:
            insts = list(bb.instructions)
            out, changed = [], False
            for inst in insts:
                si = inst.sync_info
                waits = list(si.on_wait) if si else []
                if len(waits) > max_waits:
                    for w in waits[:-max_waits]:
                        n = mybir.InstNoOp(
                            name=f"I-wsplit-{nc.next_id()}", ins=[], outs=[]
                        )
                        n.engine = inst.engine
                        n.sync_info = bass_rust.SyncInfo(on_wait=[w], on_update=[])
                        out.append(n)
                    inst.sync_info = bass_rust.SyncInfo(
                        on_wait=waits[-max_waits:], on_update=list(si.on_update)
                    )
                    changed = True
                out.append(inst)
            if changed:
                bb.instructions = out


PHASE_MARKS = {}


def _build_kernel():
    nc = bass.Bass("TRN2", target_bir_lowering=False, debug=False)
    PHASE_MARKS.clear()

    def mark(name):
        PHASE_MARKS[name] = nc.next_id()

    x_d = nc.dram_tensor("x", [NCT, 128, S], BF16, kind="ExternalInput").ap()
    # packed bf16 consts: [:, 0:76] wmT, [:, 76:204] identity,
    # row0 204:223 bm, row0 223:351 ones
    constA_d = nc.dram_tensor("constA", [128, 352], BF16, kind="ExternalInput").ap()
    wfT_d = nc.dram_tensor("wfT", [K, 128, NCT, C], F8, kind="ExternalInput").ap()
    bfT_d = nc.dram_tensor("bfT", [128, NCT * K], F32, kind="ExternalInput").ap()
    # predT[p, m, k] = pred[k, m*128 + p]: contiguous 1216B runs per partition
    predT_d = nc.dram_tensor("predT", [128, NBLK, K], F32, kind="ExternalOutput").ap()

    with TC(nc) as tc, contextlib.ExitStack() as ctx:
        const_p = ctx.enter_context(tc.tile_pool(name="const", bufs=1))
        x_p = ctx.enter_context(tc.tile_pool(name="x", bufs=1))
        maskT_p = ctx.enter_context(tc.tile_pool(name="maskT", bufs=1))
        xT_p = ctx.enter_context(tc.tile_pool(name="xT", bufs=5))
        wf_p = ctx.enter_context(tc.tile_pool(name="wf", bufs=K))
        cfT_p = ctx.enter_context(tc.tile_pool(name="cfT", bufs=1))
        fT_p = ctx.enter_context(tc.tile_pool(name="fT", bufs=1))
        stage_p = ctx.enter_context(tc.tile_pool(name="stage", bufs=4))

        # --- x slab 0 first, consts (on ACT queue) overlap, then the rest ---
        x_t = [[None] * NSLAB for _ in range(NCT)]

        def emit_x_slab(sl):
            for ct in range(NCT):
                t = x_p.tile([128, SLABW], BF16, tag=f"x{ct}_{sl}")
                nc.sync.dma_start(
                    t[:], x_d[ct, :, sl * SLABW:(sl + 1) * SLABW]
                )
                x_t[ct][sl] = t

        emit_x_slab(0)
        constA = const_p.tile([128, 352], BF16, tag="constA")
        nc.scalar.dma_start(constA[:], constA_d[:])
        wm_sb = constA[:, 0:NCT * K]
        ident_sb = constA[:, 76:204]
        bm_sb = constA[0:1, 204:204 + K]
        ones_sb = constA[0:1, 223:351]
        bf_sb = const_p.tile([128, NCT * K], F32, tag="bf")
        nc.scalar.dma_start(bf_sb[:], bfT_d[:])
        for sl in range(1, NSLAB):
            emit_x_slab(sl)

        def xslice(ct, col0, width=128):
            sl, off = divmod(col0, SLABW)
            assert off + width <= SLABW
            return x_t[ct][sl][:, off:off + width]

        # GPSIMD cannot touch PSUM; rotate PSUM->SBUF copies over DVE (2x
        # bf16 mode) and ACT, weighted toward DVE.
        copy_engines = [
            nc.vector.tensor_copy,
            nc.scalar.copy,
            nc.vector.tensor_copy,
        ]

        mark("A_start")
        # --- phase A: mask (A1) + x-transpose + class_feat (A2), per slab ---
        maskT = []   # 16 tiles [128, 8*19] bf16; (p, b*19+k) = mask[k, (8g+b)*128+p]
        with tc.tile_pool(name="ps_a1", bufs=2, space="PSUM") as a1_ps, \
             tc.tile_pool(name="ps_tr", bufs=5, space="PSUM") as tr_ps, \
             tc.tile_pool(name="ps_cf", bufs=1, space="PSUM") as cf_psp:
            # all 4 cfT accumulators share one bank: a single start marks the
            # bank pending-zero; each column group's first write clears its
            # own bytes, later writes accumulate.
            cf_ps = cf_psp.tile([128, 512], F32, tag="cf", name="cf")
            xt_tiles = {}

            def emit_a2_mm(pair):
                xt = xt_tiles.pop(pair)
                for half in range(2):
                    j = 2 * pair + half
                    g, b = divmod(j, 8)
                    for cb in range(NCT):
                        nc.tensor.matmul(
                            cf_ps[:, K * cb:K * (cb + 1)],
                            lhsT=xt[:, (4 * half + cb) * 128:
                                    (4 * half + cb + 1) * 128],
                            rhs=maskT[g][:, K * b:K * (b + 1)],
                            start=(j == 0 and cb == 0),
                            stop=(j == NBLK - 1 and cb == NCT - 1),
                            skip_group_check=True,
                        )

            GPS = NGRP // NSLAB   # A1 groups per slab
            PPS = NPAIR // NSLAB  # A2 pairs per slab
            for sl in range(NSLAB):
                # A1: mask for this slab
                for g in range(GPS * sl, GPS * (sl + 1)):
                    ps = a1_ps.tile([128, 512], F32)
                    for b in range(8):
                        blk = 8 * g + b
                        for ct in range(NCT):
                            nc.tensor.matmul(
                                ps[:, K * b:K * (b + 1)],
                                lhsT=xslice(ct, blk * 128),
                                rhs=wm_sb[:, K * ct:K * (ct + 1)],
                                start=(b == 0 and ct == 0),
                                stop=False,
                                skip_group_check=True,
                            )
                        nc.tensor.matmul(
                            ps[:, K * b:K * (b + 1)],
                            lhsT=ones_sb,
                            rhs=bm_sb,
                            start=False,
                            stop=(b == 7),
                            skip_group_check=True,
                        )
                    mt = maskT_p.tile([128, 8 * K], BF16, tag=f"mT{g}",
                                      name=f"mT{g}")
                    nc.scalar.activation(
                        mt[:], ps[:, :8 * K],
                        mybir.ActivationFunctionType.Sigmoid,
                    )
                    maskT.append(mt)
                # A2: transpose x blocks + accumulate cfT (lag pipeline)
                for pair in range(PPS * sl, PPS * (sl + 1)):
                    tp = tr_ps.tile([128, 1024], BF16)
                    for half in range(2):
                        j = 2 * pair + half
                        for cb in range(NCT):
                            nc.tensor.matmul(
                                tp[:, (4 * half + cb) * 128:
                                   (4 * half + cb + 1) * 128],
                                lhsT=xslice(cb, j * 128),
                                rhs=ident_sb,
                                is_transpose=True,
                                start=True, stop=True,
                                skip_group_check=True,
                            )
                    xt = xT_p.tile([128, 1024], BF16)
                    copy_engines[pair % 3](xt[:], tp[:])
                    xt_tiles[pair] = xt
                    if pair >= LAG:
                        emit_a2_mm(pair - LAG)
            for pair in range(NPAIR - LAG, NPAIR):
                emit_a2_mm(pair)

            # fold pooling 1/S and the fp8 Wf scale into cfT
            cfT_sb = cfT_p.tile([128, NCT * K], BF16, tag="cfT")
            nc.vector.tensor_scalar_mul(
                cfT_sb[:], cf_ps[:, :NCT * K], 1.0 / (WF_SCALE * S)
            )

        mark("B_start")
        # --- phase B: per-class filter GEMM; fT[c, k] ready for step C ---
        # all 4 psf oc-chunks share one PSUM bank (single start/stop), so the
        # bias add is a single instruction.
        with tc.tile_pool(name="ps_f", bufs=1, space="PSUM") as f_psp:
            psf = f_psp.tile([128, 512], F32, tag="psf", name="psf")
            for kk in range(K):
                wf = wf_p.tile([128, NCT, C], F8)
                nc.sync.dma_start(wf[:], wfT_d[kk])
                for ct in range(NCT):
                    for oc in range(NCT):
                        nc.tensor.matmul(
                            psf[:, K * oc + kk:K * oc + kk + 1],
                            lhsT=wf[:, ct, oc * 128:(oc + 1) * 128],
                            rhs=cfT_sb[:, K * ct + kk:K * ct + kk + 1],
                            start=(kk == 0 and ct == 0 and oc == 0),
                            stop=(kk == K - 1 and ct == NCT - 1
                                  and oc == NCT - 1),
                            skip_group_check=True,
                        )
            fT_sb = fT_p.tile([128, NCT * K], BF16, tag="fT")
            nc.vector.tensor_add(fT_sb[:], psf[:, :NCT * K], bf_sb[:])

        mark("C_start")
        # --- phase C: predT[s, k] = X^T @ fT, bank-packed like A1 ---
        pout = predT_d
        with tc.tile_pool(name="ps_c", bufs=4, space="PSUM") as c_ps:
            for gg in range(NGRP // 2):
                st = stage_p.tile([128, 16, K], F32)
                for h in range(2):
                    g = 2 * gg + h
                    ps = c_ps.tile([128, 512], F32)
                    for b in range(8):
                        blk = 8 * g + b
                        for ct in range(NCT):
                            nc.tensor.matmul(
                                ps[:, K * b:K * (b + 1)],
                                lhsT=xslice(ct, blk * 128),
                                rhs=fT_sb[:, K * ct:K * (ct + 1)],
                                start=(b == 0 and ct == 0),
                                stop=(b == 7 and ct == NCT - 1),
                                skip_group_check=True,
                            )
                    nc.vector.tensor_copy(
                        st[:, 8 * h:8 * (h + 1), :].rearrange("p a b -> p (a b)"),
                        ps[:, :8 * K],
                    )
                nc.sync.dma_start(pout[:, 16 * gg:16 * (gg + 1), :], st[:])

    mark("end")
    _split_multiwaits(nc)
    return nc


_NC_CACHE = None


def _prep_in_maps(x, Wm, bm, Wf, bf):
    x = np.asarray(x, np.float32)
    constA = np.zeros((128, 352), npbf16)
    # wmT[c_local, ct*K + k] = Wm[k, 128*ct + c_local]
    constA[:, 0:NCT * K] = (
        Wm.T.reshape(NCT, 128, K).transpose(1, 0, 2).reshape(128, NCT * K)
    ).astype(npbf16)
    constA[:, 76:204] = np.eye(128, dtype=npbf16)
    constA[0, 204:204 + K] = bm.astype(npbf16)
    constA[0, 223:351] = np.ones(128, npbf16)
    # wfT[k, c_local, ct, o] = Wf[k, o, 128*ct + c_local] * WF_SCALE (fp8;
    # the 1/(WF_SCALE*S) is folded into cfT on-chip)
    wfT = np.ascontiguousarray(
        (Wf.transpose(0, 2, 1) * WF_SCALE)
        .reshape(K, NCT, 128, C).transpose(0, 2, 1, 3)
    ).astype(npf8)
    bfT = np.ascontiguousarray(
        bf.T.reshape(NCT, 128, K).transpose(1, 0, 2).reshape(128, NCT * K)
    ).astype(np.float32)
    maps = []
    for i in range(N_CORES):
        xi = np.ascontiguousarray(
            x[i].reshape(NCT, 128, S).astype(npbf16)
        )
        maps.append({
            "x": xi,
            "constA": constA,
            "wfT": wfT,
            "bfT": bfT,
        })
    return maps


def kernel(x, Wm, bm, Wf, bf):
    global _NC_CACHE
    if _NC_CACHE is None:
        _NC_CACHE = _build_kernel()
    nc = _NC_CACHE

    in_maps = _prep_in_maps(
        np.asarray(x, np.float32), np.asarray(Wm, np.float32),
        np.asarray(bm, np.float32), np.asarray(Wf, np.float32),
        np.asarray(bf, np.float32))

    res = run_bass_kernel_spmd(nc, in_maps, list(range(N_CORES)))
    # predT[p, m, k] -> pred[k, m*128 + p] -> [K, H, W]
    out = np.stack([
        np.ascontiguousarray(res.results[i]["predT"].transpose(2, 1, 0))
        .reshape(K, H, W)
        for i in range(N_CORES)
    ])
    return out.astype(np.float32)


def time_kernel(inputs, iters=20):
    """Steady-state per-launch wall time (ns) with device-resident inputs."""
    import jax
    from jax.sharding import Mesh, PartitionSpec, NamedSharding
    from jax.experimental.shard_map import shard_map
    from concourse import mybir as _mybir
    from concourse.bass2jax import (
        _bass_exec_p, install_neuronx_cc_hook, partition_id_tensor,
    )
    import time as _time

    global _NC_CACHE
    if _NC_CACHE is None:
        _NC_CACHE = _build_kernel()
    nc = _NC_CACHE
    install_neuronx_cc_hook()

    in_maps = _prep_in_maps(
        np.asarray(inputs["x"], np.float32), np.asarray(inputs["Wm"], np.float32),
        np.asarray(inputs["bm"], np.float32), np.asarray(inputs["Wf"], np.float32),
        np.asarray(inputs["bf"], np.float32))

    in_names, out_names, out_avals, zero_outs = [], [], [], []
    pid_name = nc.partition_id_tensor.name if nc.partition_id_tensor else None
    for alloc in nc.m.functions[0].allocations:
        if not isinstance(alloc, _mybir.MemoryLocationSet):
            continue
        name = alloc.memorylocations[0].name
        if alloc.kind == "ExternalInput":
            if name != pid_name:
                in_names.append(name)
        elif alloc.kind == "ExternalOutput":
            shape = tuple(alloc.tensor_shape)
            dt = _mybir.dt.np(alloc.dtype)
            out_names.append(name)
            out_avals.append(jax.core.ShapedArray(shape, dt))
            zero_outs.append(np.zeros(shape, dt))
    n_params = len(in_names)
    all_in_names = in_names + out_names
    if nc.partition_id_tensor is not None:
        all_in_names = all_in_names + [nc.partition_id_tensor.name]

    def _body(*args):
        operands = list(args)
        if nc.partition_id_tensor is not None:
            operands.append(partition_id_tensor())
        outs = _bass_exec_p.bind(
            *operands,
            out_avals=tuple(out_avals),
            in_names=tuple(all_in_names),
            out_names=tuple(out_names),
            lowering_input_output_aliases=(),
            sim_require_finite=True,
            sim_require_nnan=True,
            nc=nc,
        )
        return tuple(outs)

    devices = jax.devices()[:N_CORES]
    mesh = Mesh(np.asarray(devices), ("core",))
    spec = PartitionSpec("core")
    n_outs = len(out_names)
    sharded = jax.jit(
        shard_map(
            _body, mesh=mesh, in_specs=(spec,) * (n_params + n_outs),
            out_specs=(spec,) * n_outs, check_rep=False,
        ),
        keep_unused=True,
    )
    concat_in = [
        np.concatenate([np.asarray(in_maps[c][nm]) for c in range(N_CORES)], axis=0)
        for nm in in_names
    ]
    concat_zeros = [
        np.zeros((N_CORES * z.shape[0], *z.shape[1:]), z.dtype) for z in zero_outs
    ]
    sh = NamedSharding(mesh, spec)
    dev_in = [jax.device_put(a, sh) for a in concat_in + concat_zeros]
    out = sharded(*dev_in)
    jax.block_until_ready(out)
    t0 = _time.perf_counter()
    for _ in range(iters):
        out = sharded(*dev_in)
    jax.block_until_ready(out)
    dt = (_time.perf_counter() - t0) / iters
    return dt * 1e9


# revision 35
# speedup vs baseline: 2.4331x; 1.0226x over previous
"""ConditionalFilterLayer Bass/Tile kernel for 8 Trainium2 NeuronCores.

Strategy: pure data parallel over batch (1 sample per core), all bf16.
Per core, with X = x[i] viewed as [C=512, S=16384] (c-major), X is loaded
from HBM exactly once (16MB bf16) and stays resident in SBUF. Every GEMM
is phrased with an X-chunk as the 128x128 *stationary* operand and a
19-column matrix as the moving operand, so the PE streams only ~19 cycles
per 128x128 contraction tile:

  A1. preT[s,k]  = X-chunk^T @ WmT-chunk (+ ones^T @ bm), sigmoid on ACT
      -> maskT produced directly in [s, k] layout (no mask transpose).
  A2. xT blocks via PE transpose (identity moving), then
      cfT[c,k] += xT-block^T @ maskT-block  (PSUM accumulated over all s)
  B.  psf[o,k]  += Wf-chunk^T @ cfT-column  (per-class filter GEMM),
      + bf -> fT (bf16)
  C.  predT[s,k] = X-chunk^T @ fT-chunk     -> HBM as [S, 19] f32

DMA traffic per core: 16MB x + 10MB Wf + 1.25MB out (vs 43MB baseline).
PSUM outputs are bank-packed (8 s-blocks of 19 cols per 2KB bank) so the
PSUM->SBUF copies are batched; copies rotate across DVE/ACT/GpSimd.
"""

import contextlib

import numpy as np
import ml_dtypes

import concourse.bass as bass
import concourse.tile as tile
from concourse import mybir
from concourse.bass_utils import run_bass_kernel_spmd
from concourse.vector_clock import ScopedClock

B, C, K, H, W = 8, 512, 19, 128, 128
S = H * W                    # 16384
NCT = C // 128               # 4 c-chunks
NSLAB = 8                    # x DMA slabs per c-chunk
SLABW = S // NSLAB           # 2048 s-cols per slab
NBLK = S // 128              # 128 s-blocks / s-chunks
NPAIR = NBLK // 2            # 64 transpose pairs
NGRP = NBLK // 8             # 16 groups of 8 s-blocks (one PSUM bank each)
LAG = 4                      # A2 software pipeline depth (pairs)
N_CORES = 8

F32 = mybir.dt.float32
BF16 = mybir.dt.bfloat16
F8 = mybir.dt.float8e4
npbf16 = ml_dtypes.bfloat16
npf8 = ml_dtypes.float8_e4m3
WF_SCALE = 16.0              # lifts Wf into fp8 e4m3 normal range


class TC(tile.TileContext):
    """TileContext whose exit drain carries at most one sync wait per
    instruction — this walrus build rejects multi-wait CTRL ops."""

    def _drain_and_barrier(self, tick_clock, wait_clock):
        nc = self.nc
        drain_inst = nc.sync.drain()
        wait_clock.add_sem_waits(
            drain_inst.ins, ScopedClock({None: tick_clock.global_clock})
        )
        si = drain_inst.ins.sync_info
        waits = list(si.on_wait) if si else []
        if len(waits) > 1:
            SyncInfo = type(si)
            drain_inst.ins.sync_info = SyncInfo(on_wait=[waits[0]], on_update=[])
            for w in waits[1:]:
                n = nc.sync.nop(nofuse=True, hint="split_drain_wait")
                n.ins.sync_info = SyncInfo(on_wait=[w], on_update=[])
        nc.all_engine_barrier()
        assert self.sems is not None
        popped = nc._tile_sem_poison_stack.pop()
        assert popped is self._sem_poison
        nc.clear_and_free_semaphores(list(self.sems.allocated().values()))
        nc.all_engine_barrier()


def _split_multiwaits(nc, max_waits=1):
    """This walrus build rejects instructions with more than one sync wait:
    peel extra waits onto same-engine no-ops inserted just before."""
    import bass_rust
    for f in nc.m.functions:
        for bb in f.blocks:
            insts = list(bb.instructions)
            out, changed = [], False
            for inst in insts:
                si = inst.sync_info
                waits = list(si.on_wait) if si else []
                if len(waits) > max_waits:
                    for w in waits[:-max_waits]:
                        n = mybir.InstNoOp(
                            name=f"I-wsplit-{nc.next_id()}", ins=[], outs=[]
                        )
                        n.engine = inst.engine
                        n.sync_info = bass_rust.SyncInfo(on_wait=[w], on_update=[])
                        out.append(n)
                    inst.sync_info = bass_rust.SyncInfo(
                        on_wait=waits[-max_waits:], on_update=list(si.on_update)
                    )
                    changed = True
                out.append(inst)
            if changed:
                bb.instructions = out


PHASE_MARKS = {}


def _build_kernel():
    nc = bass.Bass("TRN2", target_bir_lowering=False, debug=False)
    PHASE_MARKS.clear()

    def mark(name):
        PHASE_MARKS[name] = nc.next_id()

    x_d = nc.dram_tensor("x", [NCT, 128, S], BF16, kind="ExternalInput").ap()
    # packed bf16 consts: [:, 0:76] wmT, [:, 76:204] identity,
    # row0 204:223 bm, row0 223:351 ones
    constA_d = nc.dram_tensor("constA", [128, 352], BF16, kind="ExternalInput").ap()
    wfT_d = nc.dram_tensor("wfT", [K, 128, NCT, C], F8, kind="ExternalInput").ap()
    bfT_d = nc.dram_tensor("bfT", [128, NCT * K], F32, kind="ExternalInput").ap()
    # predT[p, m, k] = pred[k, m*128 + p]: contiguous 1216B runs per partition
    predT_d = nc.dram_tensor("predT", [128, NBLK, K], F32, kind="ExternalOutput").ap()

    with TC(nc) as tc, contextlib.ExitStack() as ctx:
        const_p = ctx.enter_context(tc.tile_pool(name="const", bufs=1))
        x_p = ctx.enter_context(tc.tile_pool(name="x", bufs=1))
        maskT_p = ctx.enter_context(tc.tile_pool(name="maskT", bufs=1))
        xT_p = ctx.enter_context(tc.tile_pool(name="xT", bufs=6))
        wf_p = ctx.enter_context(tc.tile_pool(name="wf", bufs=16))
        cfT_p = ctx.enter_context(tc.tile_pool(name="cfT", bufs=1))
        fT_p = ctx.enter_context(tc.tile_pool(name="fT", bufs=1))
        stage_p = ctx.enter_context(tc.tile_pool(name="stage", bufs=4))

        # --- x slab 0 first, consts (on ACT queue) overlap, then the rest ---
        x_t = [[None] * NSLAB for _ in range(NCT)]

        def emit_x_slab(sl):
            for ct in range(NCT):
                t = x_p.tile([128, SLABW], BF16, tag=f"x{ct}_{sl}")
                nc.sync.dma_start(
                    t[:], x_d[ct, :, sl * SLABW:(sl + 1) * SLABW]
                )
                x_t[ct][sl] = t

        emit_x_slab(0)
        constA = const_p.tile([128, 352], BF16, tag="constA")
        nc.scalar.dma_start(constA[:], constA_d[:])
        wm_sb = constA[:, 0:NCT * K]
        ident_sb = constA[:, 76:204]
        bm_sb = constA[0:1, 204:204 + K]
        ones_sb = constA[0:1, 223:351]
        bf_sb = const_p.tile([128, NCT * K], F32, tag="bf")
        nc.scalar.dma_start(bf_sb[:], bfT_d[:])
        for sl in range(1, NSLAB):
            emit_x_slab(sl)

        def xslice(ct, col0, width=128):
            sl, off = divmod(col0, SLABW)
            assert off + width <= SLABW
            return x_t[ct][sl][:, off:off + width]

        # GPSIMD cannot touch PSUM; rotate PSUM->SBUF copies over DVE (2x
        # bf16 mode) and ACT, weighted 3:2 toward DVE.
        copy_engines = [
            nc.vector.tensor_copy,
            nc.scalar.copy,
            nc.vector.tensor_copy,
            nc.scalar.copy,
            nc.vector.tensor_copy,
        ]

        mark("A_start")
        # --- phase A: mask (A1) + x-transpose + class_feat (A2), per slab ---
        maskT = []   # 16 tiles [128, 8*19] bf16; (p, b*19+k) = mask[k, (8g+b)*128+p]
        with tc.tile_pool(name="ps_a1", bufs=2, space="PSUM") as a1_ps, \
             tc.tile_pool(name="ps_tr", bufs=5, space="PSUM") as tr_ps, \
             tc.tile_pool(name="ps_cf", bufs=1, space="PSUM") as cf_psp:
            # all 4 cfT accumulators share one bank: a single start marks the
            # bank pending-zero; each column group's first write clears its
            # own bytes, later writes accumulate.
            cf_ps = cf_psp.tile([128, 512], F32, tag="cf", name="cf")
            xt_tiles = {}

            def emit_a2_mm(pair):
                xt = xt_tiles.pop(pair)
                for half in range(2):
                    j = 2 * pair + half
                    g, b = divmod(j, 8)
                    for cb in range(NCT):
                        nc.tensor.matmul(
                            cf_ps[:, K * cb:K * (cb + 1)],
                            lhsT=xt[:, (4 * half + cb) * 128:
                                    (4 * half + cb + 1) * 128],
                            rhs=maskT[g][:, K * b:K * (b + 1)],
                            start=(j == 0 and cb == 0),
                            stop=(j == NBLK - 1 and cb == NCT - 1),
                            skip_group_check=True,
                        )

            GPS = NGRP // NSLAB   # A1 groups per slab
            PPS = NPAIR // NSLAB  # A2 pairs per slab

            def emit_a1_slab(sl):
                for g in range(GPS * sl, GPS * (sl + 1)):
                    ps = a1_ps.tile([128, 512], F32)
                    for b in range(8):
                        blk = 8 * g + b
                        for ct in range(NCT):
                            nc.tensor.matmul(
                                ps[:, K * b:K * (b + 1)],
                                lhsT=xslice(ct, blk * 128),
                                rhs=wm_sb[:, K * ct:K * (ct + 1)],
                                start=(b == 0 and ct == 0),
                                stop=False,
                                skip_group_check=True,
                            )
                        nc.tensor.matmul(
                            ps[:, K * b:K * (b + 1)],
                            lhsT=ones_sb,
                            rhs=bm_sb,
                            start=False,
                            stop=(b == 7),
                            skip_group_check=True,
                        )
                    mt = maskT_p.tile([128, 8 * K], BF16, tag=f"mT{g}",
                                      name=f"mT{g}")
                    nc.scalar.activation(
                        mt[:], ps[:, :8 * K],
                        mybir.ActivationFunctionType.Sigmoid,
                    )
                    maskT.append(mt)

            # A1 runs one slab ahead of A2 so sigmoid latency (queued behind
            # ACT copies) never blocks the A2 matmul stream.
            emit_a1_slab(0)
            for sl in range(NSLAB):
                if sl + 1 < NSLAB:
                    emit_a1_slab(sl + 1)
                # A2: transpose x blocks + accumulate cfT (lag pipeline)
                for pair in range(PPS * sl, PPS * (sl + 1)):
                    tp = tr_ps.tile([128, 1024], BF16)
                    for half in range(2):
                        j = 2 * pair + half
                        for cb in range(NCT):
                            nc.tensor.matmul(
                                tp[:, (4 * half + cb) * 128:
                                   (4 * half + cb + 1) * 128],
                                lhsT=xslice(cb, j * 128),
                                rhs=ident_sb,
                                is_transpose=True,
                                start=True, stop=True,
                                skip_group_check=True,
                            )
                    xt = xT_p.tile([128, 1024], BF16)
                    copy_engines[pair % 5](xt[:], tp[:])
                    xt_tiles[pair] = xt
                    if pair >= LAG:
                        emit_a2_mm(pair - LAG)
            for pair in range(NPAIR - LAG, NPAIR):
                emit_a2_mm(pair)

            # fold pooling 1/S and the fp8 Wf scale into cfT
            cfT_sb = cfT_p.tile([128, NCT * K], BF16, tag="cfT")
            nc.vector.tensor_scalar_mul(
                cfT_sb[:], cf_ps[:, :NCT * K], 1.0 / (WF_SCALE * S)
            )

        mark("B_start")
        # --- phase B: per-class filter GEMM; fT[c, k] ready for step C ---
        # all 4 psf oc-chunks share one PSUM bank (single start/stop), so the
        # bias add is a single instruction.
        with tc.tile_pool(name="ps_f", bufs=1, space="PSUM") as f_psp:
            psf = f_psp.tile([128, 512], F32, tag="psf", name="psf")
            warm = f_psp.tile([128, 256], BF16, tag="warm", name="warm")

            def keep_warm(n):
                # dummy transposes hold the PE p-state at full clock while
                # the class-k matmuls pace behind the Wf DMA stream
                for _ in range(n):
                    nc.tensor.matmul(
                        warm[:, :128], lhsT=ident_sb, rhs=ident_sb,
                        is_transpose=True, start=True, stop=True,
                        skip_group_check=True,
                    )

            for kk in range(K):
                wf = wf_p.tile([128, NCT, C], F8)
                nc.sync.dma_start(wf[:], wfT_d[kk])
                for ct in range(NCT):
                    for oc in range(NCT):
                        nc.tensor.matmul(
                            psf[:, K * oc + kk:K * oc + kk + 1],
                            lhsT=wf[:, ct, oc * 128:(oc + 1) * 128],
                            rhs=cfT_sb[:, K * ct + kk:K * ct + kk + 1],
                            start=(kk == 0 and ct == 0 and oc == 0),
                            stop=(kk == K - 1 and ct == NCT - 1
                                  and oc == NCT - 1),
                            skip_group_check=True,
                        )
                if kk >= 6:
                    keep_warm(14 if kk < K - 1 else 6)
            fT_sb = fT_p.tile([128, NCT * K], BF16, tag="fT")
            nc.vector.tensor_add(fT_sb[:], psf[:, :NCT * K], bf_sb[:])

        mark("C_start")
        # --- phase C: predT[s, k] = X^T @ fT, bank-packed like A1 ---
        pout = predT_d
        c_copy = [nc.vector.tensor_copy, nc.scalar.copy]
        with tc.tile_pool(name="ps_c", bufs=4, space="PSUM") as c_ps:
            for gg in range(NGRP // 4):
                st = stage_p.tile([128, 32, K], F32)
                for hh in range(2):
                    # two 8-block groups per PSUM bank -> one batched copy
                    ps = c_ps.tile([128, 512], F32)
                    for h in range(2):
                        g = 4 * gg + 2 * hh + h
                        for b in range(8):
                            blk = 8 * g + b
                            for ct in range(NCT):
                                nc.tensor.matmul(
                                    ps[:, K * (8 * h + b):K * (8 * h + b + 1)],
                                    lhsT=xslice(ct, blk * 128),
                                    rhs=fT_sb[:, K * ct:K * (ct + 1)],
                                    start=(h == 0 and b == 0 and ct == 0),
                                    stop=(h == 1 and b == 7 and ct == NCT - 1),
                                    skip_group_check=True,
                                )
                    c_copy[hh % 2](
                        st[:, 16 * hh:16 * (hh + 1), :]
                        .rearrange("p a b -> p (a b)"),
                        ps[:, :16 * K],
                    )
                if gg < NGRP // 4 - 1:
                    nc.sync.dma_start(pout[:, 32 * gg:32 * (gg + 1), :], st[:])
                else:
                    # split the last write so the final transfer tail is short
                    nc.sync.dma_start(
                        pout[:, 32 * gg:32 * gg + 16, :], st[:, :16, :]
                    )
                    nc.sync.dma_start(
                        pout[:, 32 * gg + 16:32 * (gg + 1), :], st[:, 16:, :]
                    )

    mark("end")
    _split_multiwaits(nc)
    return nc


_NC_CACHE = None


def _prep_in_maps(x, Wm, bm, Wf, bf):
    x = np.asarray(x, np.float32)
    constA = np.zeros((128, 352), npbf16)
    # wmT[c_local, ct*K + k] = Wm[k, 128*ct + c_local]
    constA[:, 0:NCT * K] = (
        Wm.T.reshape(NCT, 128, K).transpose(1, 0, 2).reshape(128, NCT * K)
    ).astype(npbf16)
    constA[:, 76:204] = np.eye(128, dtype=npbf16)
    constA[0, 204:204 + K] = bm.astype(npbf16)
    constA[0, 223:351] = np.ones(128, npbf16)
    # wfT[k, c_local, ct, o] = Wf[k, o, 128*ct + c_local] * WF_SCALE (fp8;
    # the 1/(WF_SCALE*S) is folded into cfT on-chip)
    wfT = np.ascontiguousarray(
        (Wf.transpose(0, 2, 1) * WF_SCALE)
        .reshape(K, NCT, 128, C).transpose(0, 2, 1, 3)
    ).astype(npf8)
    bfT = np.ascontiguousarray(
        bf.T.reshape(NCT, 128, K).transpose(1, 0, 2).reshape(128, NCT * K)
    ).astype(np.float32)
    maps = []
    for i in range(N_CORES):
        xi = np.ascontiguousarray(
            x[i].reshape(NCT, 128, S).astype(npbf16)
        )
        maps.append({
            "x": xi,
            "constA": constA,
            "wfT": wfT,
            "bfT": bfT,
        })
    return maps


def kernel(x, Wm, bm, Wf, bf):
    global _NC_CACHE
    if _NC_CACHE is None:
        _NC_CACHE = _build_kernel()
    nc = _NC_CACHE

    in_maps = _prep_in_maps(
        np.asarray(x, np.float32), np.asarray(Wm, np.float32),
        np.asarray(bm, np.float32), np.asarray(Wf, np.float32),
        np.asarray(bf, np.float32))

    res = run_bass_kernel_spmd(nc, in_maps, list(range(N_CORES)))
    # predT[p, m, k] -> pred[k, m*128 + p] -> [K, H, W]
    out = np.stack([
        np.ascontiguousarray(res.results[i]["predT"].transpose(2, 1, 0))
        .reshape(K, H, W)
        for i in range(N_CORES)
    ])
    return out.astype(np.float32)


def time_kernel(inputs, iters=20):
    """Steady-state per-launch wall time (ns) with device-resident inputs."""
    import jax
    from jax.sharding import Mesh, PartitionSpec, NamedSharding
    from jax.experimental.shard_map import shard_map
    from concourse import mybir as _mybir
    from concourse.bass2jax import (
        _bass_exec_p, install_neuronx_cc_hook, partition_id_tensor,
    )
    import time as _time

    global _NC_CACHE
    if _NC_CACHE is None:
        _NC_CACHE = _build_kernel()
    nc = _NC_CACHE
    install_neuronx_cc_hook()

    in_maps = _prep_in_maps(
        np.asarray(inputs["x"], np.float32), np.asarray(inputs["Wm"], np.float32),
        np.asarray(inputs["bm"], np.float32), np.asarray(inputs["Wf"], np.float32),
        np.asarray(inputs["bf"], np.float32))

    in_names, out_names, out_avals, zero_outs = [], [], [], []
    pid_name = nc.partition_id_tensor.name if nc.partition_id_tensor else None
    for alloc in nc.m.functions[0].allocations:
        if not isinstance(alloc, _mybir.MemoryLocationSet):
            continue
        name = alloc.memorylocations[0].name
        if alloc.kind == "ExternalInput":
            if name != pid_name:
                in_names.append(name)
        elif alloc.kind == "ExternalOutput":
            shape = tuple(alloc.tensor_shape)
            dt = _mybir.dt.np(alloc.dtype)
            out_names.append(name)
            out_avals.append(jax.core.ShapedArray(shape, dt))
            zero_outs.append(np.zeros(shape, dt))
    n_params = len(in_names)
    all_in_names = in_names + out_names
    if nc.partition_id_tensor is not None:
        all_in_names = all_in_names + [nc.partition_id_tensor.name]

    def _body(*args):
        operands = list(args)
        if nc.partition_id_tensor is not None:
            operands.append(partition_id_tensor())
        outs = _bass_exec_p.bind(
            *operands,
            out_avals=tuple(out_avals),
            in_names=tuple(all_in_names),
            out_names=tuple(out_names),
            lowering_input_output_aliases=(),
            sim_require_finite=True,
            sim_require_nnan=True,
            nc=nc,
        )
        return tuple(outs)

    devices = jax.devices()[:N_CORES]
    mesh = Mesh(np.asarray(devices), ("core",))
    spec = PartitionSpec("core")
    n_outs = len(out_names)
    sharded = jax.jit(
        shard_map(
            _body, mesh=mesh, in_specs=(spec,) * (n_params + n_outs),
            out_specs=(spec,) * n_outs, check_rep=False,
        ),
        keep_unused=True,
    )
    concat_in = [
        np.concatenate([np.asarray(in_maps[c][nm]) for c in range(N_CORES)], axis=0)
        for nm in in_names
    ]
    concat_zeros = [
        np.zeros((N_CORES * z.shape[0], *z.shape[1:]), z.dtype) for z in zero_outs
    ]
    sh = NamedSharding(mesh, spec)
    dev_in = [jax.device_put(a, sh) for a in concat_in + concat_zeros]
    out = sharded(*dev_in)
    jax.block_until_ready(out)
    t0 = _time.perf_counter()
    for _ in range(iters):
        out = sharded(*dev_in)
    jax.block_until_ready(out)
    dt = (_time.perf_counter() - t0) / iters
    return dt * 1e9
